# revision 12
# baseline (speedup 1.0000x reference)
# Trainium2 Bass kernel for nn_Attention_35433480192757
#
# reference computation (b=4, c=128, h=w=64, n=h*w=4096):
#   GroupNorm(8, c) -> 1x1 conv qkv -> full [n, n] attention per batch
#   -> 1x1 conv proj -> residual add
#
# Sharding: 8 cores = 4 batches x 2 query-row halves. Each core computes the
# full k/v for its batch (cheap: the qkv matmuls are tiny) and attention for
# its 2048 query rows.
#
# Per-core layout strategy:
#   - x kept as [c=128 partitions, n] (channels on partitions).
#   - GroupNorm is folded into the qkv weights: xn = x*s_c + t_c with
#     per-channel s,t computed on device, so qkv = (W*s) @ x + (W@t + qkv_b).
#   - Scores are computed TRANSPOSED: ST[j, i] = k_j . q_i so that exp(ST)
#     (written straight to SBUF by the scalar engine, with the 1/sqrt(c)
#     scale folded into the activation) is already the [j, i] operand needed
#     for the PV matmul -- no transposes of the big probability matrix.
#   - softmax normalizer: sum_j exp(ST[j,i]) via an all-ones lhsT matmul
#     accumulated alongside the PV matmul; out rows are normalized after PV.
#   - All large matmuls use float32r (fp32 storage, fast PE mode).

import numpy as np
from contextlib import ExitStack

import concourse.bass as bass
from concourse import bacc
import concourse.tile as tile
import concourse.mybir as mybir
from concourse.bass import ts
from concourse.bass_utils import run_bass_kernel_spmd

P = 128          # partitions == channels
C = 128
N = 4096         # sequence length (h*w) per batch
NH = 2048        # query rows per core
CH = 512         # free-dim chunk (one PSUM bank of fp32)
NCH = N // CH    # 8 column chunks of x
NQCH = NH // CH  # 4 column chunks of xq
NJC = N // P     # 32 key chunks (contraction over j)
NIB = NH // CH   # 4 i-blocks per core
NUM_GROUPS = 8
GSIZE = C // NUM_GROUPS
EPS = 1e-5
SCALE = float(C) ** -0.5

F32 = mybir.dt.float32
F32R = mybir.dt.float32r
AOP = mybir.AluOpType
AFT = mybir.ActivationFunctionType


def _build_program(reps=1):
    nc = bacc.Bacc(trn_type="TRN2", num_devices=8)

    x_d = nc.dram_tensor("x", [P, N], F32R, kind="ExternalInput")
    xq_d = nc.dram_tensor("xq", [P, NH], F32R, kind="ExternalInput")
    wqT_d = nc.dram_tensor("wqT", [P, P], F32, kind="ExternalInput")
    wkT_d = nc.dram_tensor("wkT", [P, P], F32, kind="ExternalInput")
    wvT_d = nc.dram_tensor("wvT", [P, P], F32, kind="ExternalInput")
    wpT_d = nc.dram_tensor("wpT", [P, P], F32R, kind="ExternalInput")
    qkvb_d = nc.dram_tensor("qkvb", [P, 3], F32, kind="ExternalInput")
    pb_d = nc.dram_tensor("pb", [P, 1], F32, kind="ExternalInput")
    gnw_d = nc.dram_tensor("gnw", [P, 1], F32, kind="ExternalInput")
    gnb_d = nc.dram_tensor("gnb", [P, 1], F32, kind="ExternalInput")
    idt_d = nc.dram_tensor("idt", [P, P], F32R, kind="ExternalInput")
    out_d = nc.dram_tensor("out", [P, NH], F32, kind="ExternalOutput")

    # constants baked into the NEFF
    gmat_np = np.zeros((P, P), dtype=np.float32)
    for g in range(NUM_GROUPS):
        gmat_np[g * GSIZE:(g + 1) * GSIZE, g * GSIZE:(g + 1) * GSIZE] = 1.0 / GSIZE
    gmat_d = nc.inline_tensor(gmat_np, "gmat")

    with ExitStack() as ctx:
        tc = ctx.enter_context(tile.TileContext(nc))

        consts = ctx.enter_context(tc.tile_pool(name="consts", bufs=1))
        bigs = ctx.enter_context(tc.tile_pool(name="bigs", bufs=1))
        work = ctx.enter_context(tc.tile_pool(name="work", bufs=2))
        small = ctx.enter_context(tc.tile_pool(name="small", bufs=1))
        outp = ctx.enter_context(tc.tile_pool(name="outp", bufs=2))
        psb = ctx.enter_context(tc.tile_pool(name="psb", bufs=3, space="PSUM"))
        psacc = ctx.enter_context(tc.tile_pool(name="psacc", bufs=1, space="PSUM"))
        pssum = ctx.enter_context(tc.tile_pool(name="pssum", bufs=1, space="PSUM"))
        pstiny = pssum  # tiny preamble psum shares the "sp" slot
        pools = (consts, bigs, work, small, outp, psb, psacc, pssum, pstiny)
        drams = (x_d, xq_d, wqT_d, wkT_d, wvT_d, wpT_d, qkvb_d, pb_d,
                 gnw_d, gnb_d, idt_d, gmat_d, out_d)

        for _rep in range(reps):
            _emit_body(nc, pools, drams)

    nc.compile()
    return nc


def _emit_body(nc, pools, drams):
    (consts, bigs, work, small, outp, psb, psacc, pssum, pstiny) = pools
    (x_d, xq_d, wqT_d, wkT_d, wvT_d, wpT_d, qkvb_d, pb_d,
     gnw_d, gnb_d, idt_d, gmat_d, out_d) = drams

    # ---------------- loads ----------------
    x_sb = bigs.tile([P, N], F32R, tag="x", name="x_sb")
    for s in range(NCH):
        nc.sync.dma_start(x_sb[:, ts(s, CH)], x_d.ap()[:, ts(s, CH)])
    xq_sb = bigs.tile([P, NH], F32R, tag="xq", name="xq_sb")
    for s in range(NQCH):
        nc.sync.dma_start(xq_sb[:, ts(s, CH)], xq_d.ap()[:, ts(s, CH)])

    wq = consts.tile([P, P], F32, tag="wq", name="wq")
    nc.sync.dma_start(wq[:], wqT_d.ap())
    wk = consts.tile([P, P], F32, tag="wk", name="wk")
    nc.sync.dma_start(wk[:], wkT_d.ap())
    wv = consts.tile([P, P], F32, tag="wv", name="wv")
    nc.sync.dma_start(wv[:], wvT_d.ap())
    wp = consts.tile([P, P], F32R, tag="wp", name="wp")
    nc.sync.dma_start(wp[:], wpT_d.ap())
    idt = consts.tile([P, P], F32R, tag="idt", name="idt")
    nc.sync.dma_start(idt[:], idt_d.ap())
    qkvb = consts.tile([P, 3], F32, tag="qkvb", name="qkvb")
    nc.sync.dma_start(qkvb[:], qkvb_d.ap())
    pb = consts.tile([P, 1], F32, tag="pb", name="pb")
    nc.sync.dma_start(pb[:], pb_d.ap())
    gnw = consts.tile([P, 1], F32, tag="gnw", name="gnw")
    nc.sync.dma_start(gnw[:], gnw_d.ap())
    gnb = consts.tile([P, 1], F32, tag="gnb", name="gnb")
    nc.sync.dma_start(gnb[:], gnb_d.ap())
    ones_f = consts.tile([P, P], F32, tag="ones_f", name="ones_f")
    nc.vector.memset(ones_f[:], 1.0)
    ones = consts.tile([P, P], F32R, tag="ones", name="ones")
    nc.vector.tensor_copy(ones[:], ones_f[:])
    # gmat is DMA'd last; the warmup matmul below then observes the DMA-queue
    # semaphore once, so later matmuls need at most one wait (walrus codegen
    # allows only one sync-wait on a self-loading fp32 matmul).
    gmat = consts.tile([P, P], F32, tag="gmat", name="gmat")
    nc.sync.dma_start(gmat[:], gmat_d.ap())

    # PE warmup: absorb the DMA semaphore wait (see note above).
    ps_t = pstiny.tile([P, 8], F32, tag="sp", name="ps_t")
    nc.tensor.matmul(ps_t[:, 6:8], lhsT=gmat[:], rhs=gmat[:, 0:2])

    # ---------------- GroupNorm stats ----------------
    stats = small.tile([P, NCH, 6], F32, tag="stats", name="stats")
    for s in range(NCH):
        nc.vector.bn_stats(stats[:, s, :], x_sb[:, ts(s, CH)])
    mv = small.tile([P, 2], F32, tag="mv", name="mv")  # per-channel mean, var
    nc.vector.bn_aggr(mv[:], stats[:])

    # t2 = [mean_c, E[x^2]_c]
    t2 = small.tile([P, 2], F32, tag="t2", name="t2")
    nc.vector.tensor_mul(t2[:, 1:2], mv[:, 0:1], mv[:, 0:1])
    nc.vector.tensor_add(t2[:, 1:2], t2[:, 1:2], mv[:, 1:2])
    nc.vector.tensor_copy(t2[:, 0:1], mv[:, 0:1])

    # group-average both stats with the block-diagonal averaging matrix
    nc.tensor.matmul(ps_t[:, 0:2], lhsT=gmat[:], rhs=t2[:])
    gstat = small.tile([P, 2], F32, tag="gstat", name="gstat")
    nc.vector.tensor_copy(gstat[:], ps_t[:, 0:2])

    varv = small.tile([P, 1], F32, tag="varv", name="varv")  # var_g + eps
    nc.vector.tensor_mul(varv[:], gstat[:, 0:1], gstat[:, 0:1])
    nc.vector.tensor_sub(varv[:], gstat[:, 1:2], varv[:])
    nc.vector.tensor_scalar_add(varv[:], varv[:], EPS)

    rstd = small.tile([P, 1], F32, tag="rstd", name="rstd")
    lnv = small.tile([P, 1], F32, tag="lnv", name="lnv")
    nc.scalar.activation(lnv[:], varv[:], AFT.Ln)
    nc.scalar.activation(rstd[:], lnv[:], AFT.Exp, scale=-0.5)
    # two Newton steps to clean up the ACT sqrt approximation:
    # y <- y * (1.5 - 0.5 * v * y * y)
    for it in range(2):
        nt = small.tile([P, 1], F32, tag="nt", name=f"nt{it}")
        nc.vector.tensor_mul(nt[:], rstd[:], rstd[:])
        nc.vector.tensor_mul(nt[:], nt[:], varv[:])
        nc.vector.tensor_scalar(nt[:], nt[:], -0.5, 1.5, AOP.mult, AOP.add)
        nc.vector.tensor_mul(rstd[:], rstd[:], nt[:])

    s_c = small.tile([P, 1], F32, tag="s_c", name="s_c")  # per-channel scale
    nc.vector.tensor_mul(s_c[:], rstd[:], gnw[:])
    t_c = small.tile([P, 1], F32, tag="t_c", name="t_c")  # per-channel shift
    nc.vector.tensor_mul(t_c[:], gstat[:, 0:1], s_c[:])
    nc.vector.tensor_sub(t_c[:], gnb[:], t_c[:])

    # ---------------- fold GN into qkv weights ----------------
    wq_s = consts.tile([P, P], F32R, tag="wq_s", name="wq_s")
    nc.vector.tensor_scalar_mul(wq_s[:], wq[:], s_c[:])
    wk_s = consts.tile([P, P], F32R, tag="wk_s", name="wk_s")
    nc.vector.tensor_scalar_mul(wk_s[:], wk[:], s_c[:])
    wv_s = consts.tile([P, P], F32R, tag="wv_s", name="wv_s")
    nc.vector.tensor_scalar_mul(wv_s[:], wv[:], s_c[:])

    # bias_{q,k,v}[o] = sum_c W[o,c] * t_c + qkv_bias[o]
    nc.tensor.matmul(ps_t[:, 2:3], lhsT=wq[:], rhs=t_c[:])
    nc.tensor.matmul(ps_t[:, 3:4], lhsT=wk[:], rhs=t_c[:])
    nc.tensor.matmul(ps_t[:, 4:5], lhsT=wv[:], rhs=t_c[:])
    bq = small.tile([P, 1], F32, tag="bq", name="bq")
    nc.vector.tensor_add(bq[:], ps_t[:, 2:3], qkvb[:, 0:1])
    bk = small.tile([P, 1], F32, tag="bk", name="bk")
    nc.vector.tensor_add(bk[:], ps_t[:, 3:4], qkvb[:, 1:2])
    bv = small.tile([P, 1], F32, tag="bv", name="bv")
    nc.vector.tensor_add(bv[:], ps_t[:, 4:5], qkvb[:, 2:3])

    # ---------------- qkv projections ----------------
    kT_sb = bigs.tile([P, NCH, CH], F32R, tag="kT", name="kT_sb")
    vT_sb = bigs.tile([P, NCH, CH], F32R, tag="vT", name="vT_sb")
    qT_sb = bigs.tile([P, NQCH, CH], F32R, tag="qT", name="qT_sb")
    for s in range(NCH):
        psk = psb.tile([P, 2, CH], F32, tag="sc", name=f"psk{s}")
        nc.tensor.matmul(psk[:, 0, :], lhsT=wk_s[:], rhs=x_sb[:, ts(s, CH)])
        nc.vector.tensor_scalar(kT_sb[:, s, :], psk[:, 0, :], bk[:], None,
                                AOP.add)
    for s in range(NQCH):
        psq = psb.tile([P, 2, CH], F32, tag="sc", name=f"psq{s}")
        nc.tensor.matmul(psq[:, 0, :], lhsT=wq_s[:], rhs=xq_sb[:, ts(s, CH)])
        nc.vector.tensor_scalar(qT_sb[:, s, :], psq[:, 0, :], bq[:], None,
                                AOP.add)
    for s in range(NCH):
        psv = psb.tile([P, 2, CH], F32, tag="sc", name=f"psv{s}")
        nc.tensor.matmul(psv[:, 0, :], lhsT=wv_s[:], rhs=x_sb[:, ts(s, CH)])
        nc.vector.tensor_scalar(vT_sb[:, s, :], psv[:, 0, :], bv[:], None, AOP.add)

    # ---------------- v to natural [j, c] layout ----------------
    vnat_sb = bigs.tile([P, NJC, P], F32R, tag="vnat", name="vnat_sb")
    for jc in range(NJC):
        src = vT_sb[:, jc // 4, (jc % 4) * P:(jc % 4 + 1) * P]
        pst = psb.tile([P, P], F32R, tag="sc", name=f"pst{jc}")
        nc.tensor.transpose(pst[:], src, idt[:])
        nc.vector.tensor_copy(vnat_sb[:, jc, :], pst[:])

    # ---------------- attention ----------------
    PT_sb = bigs.tile([P, NJC, CH], F32R, tag="PT", name="PT_sb")

    for ib in range(NIB):
        acc = psacc.tile([P, CH], F32, tag="acc", name=f"acc{ib}")
        sm = pssum.tile([P, CH], F32, tag="sp", name=f"sm{ib}")
        qblk = qT_sb[:, ib, :]

        def emit_pv(g):
            for h in range(2):
                jc = 2 * g + h
                pslice = PT_sb[:, jc, :]
                nc.tensor.matmul(
                    acc[:], lhsT=vnat_sb[:, jc, :], rhs=pslice,
                    start=(jc == 0), stop=(jc == NJC - 1),
                    skip_group_check=True,
                )
                nc.tensor.matmul(
                    sm[:], lhsT=ones[:], rhs=pslice,
                    start=(jc == 0), stop=(jc == NJC - 1),
                    skip_group_check=True,
                )

        for g in range(NJC // 2):
            ps = psb.tile([P, 2, CH], F32, tag="sc", name=f"ps{ib}_{g}")
            for h in range(2):
                jc = 2 * g + h
                kslice = kT_sb[:, jc // 4, (jc % 4) * P:(jc % 4 + 1) * P]
                nc.tensor.matmul(ps[:, h, :], lhsT=kslice, rhs=qblk,
                                 skip_group_check=True)
            if g > 0:
                emit_pv(g - 1)
            nc.scalar.activation(PT_sb[:, 2 * g:2 * g + 2, :], ps[:],
                                 AFT.Exp, scale=SCALE)
        emit_pv(NJC // 2 - 1)

        # normalize and project
        recip = work.tile([P, CH], F32, tag="recip", name=f"recip{ib}")
        rscr = work.tile([P, CH], F32, tag="rscr", name=f"rscr{ib}")
        nc.vector.reciprocal_approx_accurate(recip[:], sm[:], rscr[:])
        outn = work.tile([P, CH], F32R, tag="outn", name=f"outn{ib}")
        nc.vector.tensor_mul(outn[:], acc[:], recip[:])

        psp = pssum.tile([P, CH], F32, tag="sp", name=f"psp{ib}")
        nc.tensor.matmul(psp[:], lhsT=wp[:], rhs=outn[:])
        stage = outp.tile([P, CH], F32, tag="stage", name=f"stage{ib}")
        nc.vector.scalar_tensor_tensor(stage[:], psp[:], pb[:, 0:1],
                                       xq_sb[:, ts(ib, CH)], AOP.add, AOP.add)
        nc.sync.dma_start(out_d.ap()[:, ts(ib, CH)], stage[:])


_NC_CACHE = {}


def _get_nc(reps=1):
    if reps not in _NC_CACHE:
        _NC_CACHE[reps] = _build_program(reps)
    return _NC_CACHE[reps]


def _make_in_maps(x, gn_weight, gn_bias, qkv_weight, qkv_bias, proj_weight,
                  proj_bias):
    x = np.ascontiguousarray(x, dtype=np.float32)
    qkv_weight = np.asarray(qkv_weight, dtype=np.float32)
    qkv_bias = np.asarray(qkv_bias, dtype=np.float32)
    proj_weight = np.asarray(proj_weight, dtype=np.float32)
    proj_bias = np.asarray(proj_bias, dtype=np.float32)
    gn_weight = np.asarray(gn_weight, dtype=np.float32)
    gn_bias = np.asarray(gn_bias, dtype=np.float32)

    b = x.shape[0]
    xf = x.reshape(b, C, N)
    wqT = np.ascontiguousarray(qkv_weight[0:C].T)
    wkT = np.ascontiguousarray(qkv_weight[C:2 * C].T)
    wvT = np.ascontiguousarray(qkv_weight[2 * C:3 * C].T)
    wpT = np.ascontiguousarray(proj_weight.T)
    qkvb = np.ascontiguousarray(qkv_bias.reshape(3, C).T)
    pbv = np.ascontiguousarray(proj_bias.reshape(C, 1))
    idt_np = np.eye(C, dtype=np.float32)
    gnwv = np.ascontiguousarray(gn_weight.reshape(C, 1))
    gnbv = np.ascontiguousarray(gn_bias.reshape(C, 1))

    in_maps = []
    for core in range(8):
        bi, half = core // 2, core % 2
        in_maps.append({
            "x": np.ascontiguousarray(xf[bi]),
            "xq": np.ascontiguousarray(xf[bi][:, half * NH:(half + 1) * NH]),
            "wqT": wqT, "wkT": wkT, "wvT": wvT, "wpT": wpT,
            "qkvb": qkvb, "pb": pbv, "gnw": gnwv, "gnb": gnbv,
            "idt": idt_np,
        })
    return in_maps


def run_on_cores(trace=False, reps=1, **inputs):
    """Build + run on the 8 cores; returns (BassKernelResults, output array)."""
    nc = _get_nc(reps)
    in_maps = _make_in_maps(**inputs)
    res = run_bass_kernel_spmd(nc, in_maps, core_ids=list(range(8)),
                               trace=trace)
    b = np.asarray(inputs["x"]).shape[0]
    h = w = 64
    out = np.empty((b, C, N), dtype=np.float32)
    for core in range(8):
        bi, half = core // 2, core % 2
        out[bi][:, half * NH:(half + 1) * NH] = res.results[core]["out"]
    return res, out.reshape(b, C, h, w)


def kernel(**inputs) -> np.ndarray:
    _, out = run_on_cores(trace=False, **inputs)
    return out


# revision 13
# speedup vs baseline: 1.1740x; 1.1740x over previous
# Trainium2 Bass kernel for nn_Attention_35433480192757
#
# reference computation (b=4, c=128, h=w=64, n=h*w=4096):
#   GroupNorm(8, c) -> 1x1 conv qkv -> full [n, n] attention per batch
#   -> 1x1 conv proj -> residual add
#
# Sharding: 8 cores = 4 batches x 2 query-row halves. Each core computes the
# full k/v for its batch (cheap: the qkv matmuls are tiny) and attention for
# its 2048 query rows.
#
# Per-core layout strategy:
#   - x kept as [c=128 partitions, n] (channels on partitions).
#   - GroupNorm is folded into the qkv weights: xn = x*s_c + t_c with
#     per-channel s,t computed on device, so qkv = (W*s) @ x + (W@t + qkv_b).
#   - Scores are computed TRANSPOSED: ST[j, i] = k_j . q_i so that exp(ST)
#     (written straight to SBUF by the scalar engine, with the 1/sqrt(c)
#     scale folded into the activation) is already the [j, i] operand needed
#     for the PV matmul -- no transposes of the big probability matrix.
#   - softmax normalizer: sum_j exp(ST[j,i]) via an all-ones lhsT matmul
#     accumulated alongside the PV matmul; out rows are normalized after PV.
#   - All large matmuls use float32r (fp32 storage, fast PE mode).

import numpy as np
from contextlib import ExitStack

import concourse.bass as bass
from concourse import bacc
import concourse.tile as tile
import concourse.mybir as mybir
from concourse.bass import ts
from concourse.bass_utils import run_bass_kernel_spmd

P = 128          # partitions == channels
C = 128
N = 4096         # sequence length (h*w) per batch
NH = 2048        # query rows per core
CH = 512         # free-dim chunk (one PSUM bank of fp32)
NCH = N // CH    # 8 column chunks of x
NQCH = NH // CH  # 4 column chunks of xq
NJC = N // P     # 32 key chunks (contraction over j)
NIB = NH // CH   # 4 i-blocks per core
NUM_GROUPS = 8
GSIZE = C // NUM_GROUPS
EPS = 1e-5
SCALE = float(C) ** -0.5

F32 = mybir.dt.float32
F32R = mybir.dt.float32r
AOP = mybir.AluOpType
AFT = mybir.ActivationFunctionType


def _build_program(reps=1):
    nc = bacc.Bacc(trn_type="TRN2", num_devices=8)

    x_d = nc.dram_tensor("x", [P, N], F32R, kind="ExternalInput")
    xq_d = nc.dram_tensor("xq", [P, NH], F32R, kind="ExternalInput")
    wqT_d = nc.dram_tensor("wqT", [P, P], F32, kind="ExternalInput")
    wkT_d = nc.dram_tensor("wkT", [P, P], F32, kind="ExternalInput")
    wvT_d = nc.dram_tensor("wvT", [P, P], F32, kind="ExternalInput")
    wpT_d = nc.dram_tensor("wpT", [P, P], F32R, kind="ExternalInput")
    qkvb_d = nc.dram_tensor("qkvb", [P, 3], F32, kind="ExternalInput")
    pb_d = nc.dram_tensor("pb", [P, 1], F32, kind="ExternalInput")
    gnw_d = nc.dram_tensor("gnw", [P, 1], F32, kind="ExternalInput")
    gnb_d = nc.dram_tensor("gnb", [P, 1], F32, kind="ExternalInput")
    idt_d = nc.dram_tensor("idt", [P, P], F32R, kind="ExternalInput")
    out_d = nc.dram_tensor("out", [P, NH], F32, kind="ExternalOutput")

    # constants baked into the NEFF
    gmat_np = np.zeros((P, P), dtype=np.float32)
    for g in range(NUM_GROUPS):
        gmat_np[g * GSIZE:(g + 1) * GSIZE, g * GSIZE:(g + 1) * GSIZE] = 1.0 / GSIZE
    gmat_d = nc.inline_tensor(gmat_np, "gmat")

    with ExitStack() as ctx:
        tc = ctx.enter_context(tile.TileContext(nc))

        consts = ctx.enter_context(tc.tile_pool(name="consts", bufs=1))
        bigs = ctx.enter_context(tc.tile_pool(name="bigs", bufs=1))
        work = ctx.enter_context(tc.tile_pool(name="work", bufs=2))
        small = ctx.enter_context(tc.tile_pool(name="small", bufs=1))
        outp = ctx.enter_context(tc.tile_pool(name="outp", bufs=2))
        psb = ctx.enter_context(tc.tile_pool(name="psb", bufs=3, space="PSUM"))
        psacc = ctx.enter_context(tc.tile_pool(name="psacc", bufs=1, space="PSUM"))
        pssum = ctx.enter_context(tc.tile_pool(name="pssum", bufs=1, space="PSUM"))
        pstiny = pssum  # tiny preamble psum shares the "sp" slot
        pools = (consts, bigs, work, small, outp, psb, psacc, pssum, pstiny)
        drams = (x_d, xq_d, wqT_d, wkT_d, wvT_d, wpT_d, qkvb_d, pb_d,
                 gnw_d, gnb_d, idt_d, gmat_d, out_d)

        for _rep in range(reps):
            _emit_body(nc, pools, drams)

    nc.compile()
    return nc


def _emit_body(nc, pools, drams):
    (consts, bigs, work, small, outp, psb, psacc, pssum, pstiny) = pools
    (x_d, xq_d, wqT_d, wkT_d, wvT_d, wpT_d, qkvb_d, pb_d,
     gnw_d, gnb_d, idt_d, gmat_d, out_d) = drams

    # ---------------- loads ----------------
    x_sb = bigs.tile([P, N], F32R, tag="x", name="x_sb")
    for s in range(NCH):
        nc.sync.dma_start(x_sb[:, ts(s, CH)], x_d.ap()[:, ts(s, CH)])
    xq_sb = bigs.tile([P, NH], F32R, tag="xq", name="xq_sb")
    for s in range(NQCH):
        nc.sync.dma_start(xq_sb[:, ts(s, CH)], xq_d.ap()[:, ts(s, CH)])

    wq = consts.tile([P, P], F32, tag="wq", name="wq")
    nc.sync.dma_start(wq[:], wqT_d.ap())
    wk = consts.tile([P, P], F32, tag="wk", name="wk")
    nc.sync.dma_start(wk[:], wkT_d.ap())
    wv = consts.tile([P, P], F32, tag="wv", name="wv")
    nc.sync.dma_start(wv[:], wvT_d.ap())
    wp = consts.tile([P, P], F32R, tag="wp", name="wp")
    nc.sync.dma_start(wp[:], wpT_d.ap())
    idt = consts.tile([P, P], F32R, tag="idt", name="idt")
    nc.sync.dma_start(idt[:], idt_d.ap())
    qkvb = consts.tile([P, 3], F32, tag="qkvb", name="qkvb")
    nc.sync.dma_start(qkvb[:], qkvb_d.ap())
    pb = consts.tile([P, 1], F32, tag="pb", name="pb")
    nc.sync.dma_start(pb[:], pb_d.ap())
    gnw = consts.tile([P, 1], F32, tag="gnw", name="gnw")
    nc.sync.dma_start(gnw[:], gnw_d.ap())
    gnb = consts.tile([P, 1], F32, tag="gnb", name="gnb")
    nc.sync.dma_start(gnb[:], gnb_d.ap())
    ones_f = consts.tile([P, P], F32, tag="ones_f", name="ones_f")
    nc.vector.memset(ones_f[:], 1.0)
    ones = consts.tile([P, P], F32R, tag="ones", name="ones")
    nc.vector.tensor_copy(ones[:], ones_f[:])
    # gmat is DMA'd last; the warmup matmul below then observes the DMA-queue
    # semaphore once, so later matmuls need at most one wait (walrus codegen
    # allows only one sync-wait on a self-loading fp32 matmul).
    gmat = consts.tile([P, P], F32, tag="gmat", name="gmat")
    nc.sync.dma_start(gmat[:], gmat_d.ap())

    # PE warmup: absorb the DMA semaphore wait (see note above).
    ps_t = pstiny.tile([P, 8], F32, tag="sp", name="ps_t")
    nc.tensor.matmul(ps_t[:, 6:8], lhsT=gmat[:], rhs=gmat[:, 0:2])

    # ---------------- GroupNorm stats ----------------
    stats = small.tile([P, NCH, 6], F32, tag="stats", name="stats")
    for s in range(NCH):
        nc.vector.bn_stats(stats[:, s, :], x_sb[:, ts(s, CH)])
    mv = small.tile([P, 2], F32, tag="mv", name="mv")  # per-channel mean, var
    nc.vector.bn_aggr(mv[:], stats[:])

    # t2 = [mean_c, E[x^2]_c]
    t2 = small.tile([P, 2], F32, tag="t2", name="t2")
    nc.vector.tensor_mul(t2[:, 1:2], mv[:, 0:1], mv[:, 0:1])
    nc.vector.tensor_add(t2[:, 1:2], t2[:, 1:2], mv[:, 1:2])
    nc.vector.tensor_copy(t2[:, 0:1], mv[:, 0:1])

    # group-average both stats with the block-diagonal averaging matrix
    nc.tensor.matmul(ps_t[:, 0:2], lhsT=gmat[:], rhs=t2[:])
    gstat = small.tile([P, 2], F32, tag="gstat", name="gstat")
    nc.vector.tensor_copy(gstat[:], ps_t[:, 0:2])

    varv = small.tile([P, 1], F32, tag="varv", name="varv")  # var_g + eps
    nc.vector.tensor_mul(varv[:], gstat[:, 0:1], gstat[:, 0:1])
    nc.vector.tensor_sub(varv[:], gstat[:, 1:2], varv[:])
    nc.vector.tensor_scalar_add(varv[:], varv[:], EPS)

    rstd = small.tile([P, 1], F32, tag="rstd", name="rstd")
    lnv = small.tile([P, 1], F32, tag="lnv", name="lnv")
    nc.scalar.activation(lnv[:], varv[:], AFT.Ln)
    nc.scalar.activation(rstd[:], lnv[:], AFT.Exp, scale=-0.5)
    # two Newton steps to clean up the ACT sqrt approximation:
    # y <- y * (1.5 - 0.5 * v * y * y)
    for it in range(2):
        nt = small.tile([P, 1], F32, tag="nt", name=f"nt{it}")
        nc.vector.tensor_mul(nt[:], rstd[:], rstd[:])
        nc.vector.tensor_mul(nt[:], nt[:], varv[:])
        nc.vector.tensor_scalar(nt[:], nt[:], -0.5, 1.5, AOP.mult, AOP.add)
        nc.vector.tensor_mul(rstd[:], rstd[:], nt[:])

    s_c = small.tile([P, 1], F32, tag="s_c", name="s_c")  # per-channel scale
    nc.vector.tensor_mul(s_c[:], rstd[:], gnw[:])
    t_c = small.tile([P, 1], F32, tag="t_c", name="t_c")  # per-channel shift
    nc.vector.tensor_mul(t_c[:], gstat[:, 0:1], s_c[:])
    nc.vector.tensor_sub(t_c[:], gnb[:], t_c[:])

    # ---------------- fold GN into qkv weights ----------------
    wq_s = consts.tile([P, P], F32R, tag="wq_s", name="wq_s")
    nc.vector.tensor_scalar_mul(wq_s[:], wq[:], s_c[:])
    wk_s = consts.tile([P, P], F32R, tag="wk_s", name="wk_s")
    nc.vector.tensor_scalar_mul(wk_s[:], wk[:], s_c[:])
    wv_s = consts.tile([P, P], F32R, tag="wv_s", name="wv_s")
    nc.vector.tensor_scalar_mul(wv_s[:], wv[:], s_c[:])

    # bias_{q,k,v}[o] = sum_c W[o,c] * t_c + qkv_bias[o]
    nc.tensor.matmul(ps_t[:, 2:3], lhsT=wq[:], rhs=t_c[:])
    nc.tensor.matmul(ps_t[:, 3:4], lhsT=wk[:], rhs=t_c[:])
    nc.tensor.matmul(ps_t[:, 4:5], lhsT=wv[:], rhs=t_c[:])
    bq = small.tile([P, 1], F32, tag="bq", name="bq")
    nc.vector.tensor_add(bq[:], ps_t[:, 2:3], qkvb[:, 0:1])
    bk = small.tile([P, 1], F32, tag="bk", name="bk")
    nc.vector.tensor_add(bk[:], ps_t[:, 3:4], qkvb[:, 1:2])
    bv = small.tile([P, 1], F32, tag="bv", name="bv")
    nc.vector.tensor_add(bv[:], ps_t[:, 4:5], qkvb[:, 2:3])

    # ---------------- qkv projections ----------------
    kT_sb = bigs.tile([P, NCH, CH], F32R, tag="kT", name="kT_sb")
    qT_sb = bigs.tile([P, NQCH, CH], F32R, tag="qT", name="qT_sb")
    for s in range(NCH):
        psk = psb.tile([P, 2, CH], F32, tag="sc", name=f"psk{s}")
        nc.tensor.matmul(psk[:, 0, :], lhsT=wk_s[:], rhs=x_sb[:, ts(s, CH)])
        nc.vector.tensor_scalar(kT_sb[:, s, :], psk[:, 0, :], bk[:], None,
                                AOP.add)
    for s in range(NQCH):
        psq = psb.tile([P, 2, CH], F32, tag="sc", name=f"psq{s}")
        nc.tensor.matmul(psq[:, 0, :], lhsT=wq_s[:], rhs=xq_sb[:, ts(s, CH)])
        nc.vector.tensor_scalar(qT_sb[:, s, :], psq[:, 0, :], bq[:], None,
                                AOP.add)
    # ---------------- v directly in natural [j, c] layout ----------------
    # v_nat[n, c] = sum_c' x[c', n] * wv_s[c', c]; the v bias is applied to
    # the normalized attention output instead (sum_j P(v+b)/sum_j P = Pv/s+b).
    vnat_sb = bigs.tile([P, NJC, P], F32R, tag="vnat", name="vnat_sb")
    for jc in range(NJC):
        psv = psb.tile([P, 2, CH], F32, tag="sc", name=f"psv{jc}")
        nc.tensor.matmul(psv[:, 0, 0:P], lhsT=x_sb[:, jc * P:(jc + 1) * P],
                         rhs=wv_s[:])
        nc.vector.tensor_copy(vnat_sb[:, jc, :], psv[:, 0, 0:P])

    # ---------------- attention ----------------
    PT_sb = bigs.tile([P, NJC, CH], F32R, tag="PT", name="PT_sb")

    for ib in range(NIB):
        acc = psacc.tile([P, CH], F32, tag="acc", name=f"acc{ib}")
        sm = pssum.tile([P, CH], F32, tag="sp", name=f"sm{ib}")
        qblk = qT_sb[:, ib, :]

        def emit_pv(g):
            for h in range(2):
                jc = 2 * g + h
                pslice = PT_sb[:, jc, :]
                nc.tensor.matmul(
                    acc[:], lhsT=vnat_sb[:, jc, :], rhs=pslice,
                    start=(jc == 0), stop=(jc == NJC - 1),
                    skip_group_check=True,
                )
                nc.tensor.matmul(
                    sm[:], lhsT=ones[:], rhs=pslice,
                    start=(jc == 0), stop=(jc == NJC - 1),
                    skip_group_check=True,
                )

        for g in range(NJC // 2):
            ps = psb.tile([P, 2, CH], F32, tag="sc", name=f"ps{ib}_{g}")
            for h in range(2):
                jc = 2 * g + h
                kslice = kT_sb[:, jc // 4, (jc % 4) * P:(jc % 4 + 1) * P]
                nc.tensor.matmul(ps[:, h, :], lhsT=kslice, rhs=qblk,
                                 skip_group_check=True)
            if g > 0:
                emit_pv(g - 1)
            nc.scalar.activation(PT_sb[:, 2 * g:2 * g + 2, :], ps[:],
                                 AFT.Exp, scale=SCALE)
        emit_pv(NJC // 2 - 1)

        # normalize and project
        recip = work.tile([P, CH], F32, tag="recip", name=f"recip{ib}")
        rscr = work.tile([P, CH], F32, tag="rscr", name=f"rscr{ib}")
        nc.vector.reciprocal_approx_accurate(recip[:], sm[:], rscr[:])
        outn = work.tile([P, CH], F32R, tag="outn", name=f"outn{ib}")
        nc.vector.tensor_mul(outn[:], acc[:], recip[:])
        nc.vector.tensor_scalar(outn[:], outn[:], bv[:], None, AOP.add)

        psp = pssum.tile([P, CH], F32, tag="sp", name=f"psp{ib}")
        nc.tensor.matmul(psp[:], lhsT=wp[:], rhs=outn[:])
        stage = outp.tile([P, CH], F32, tag="stage", name=f"stage{ib}")
        nc.vector.scalar_tensor_tensor(stage[:], psp[:], pb[:, 0:1],
                                       xq_sb[:, ts(ib, CH)], AOP.add, AOP.add)
        nc.sync.dma_start(out_d.ap()[:, ts(ib, CH)], stage[:])


_NC_CACHE = {}


def _get_nc(reps=1):
    if reps not in _NC_CACHE:
        _NC_CACHE[reps] = _build_program(reps)
    return _NC_CACHE[reps]


def _make_in_maps(x, gn_weight, gn_bias, qkv_weight, qkv_bias, proj_weight,
                  proj_bias):
    x = np.ascontiguousarray(x, dtype=np.float32)
    qkv_weight = np.asarray(qkv_weight, dtype=np.float32)
    qkv_bias = np.asarray(qkv_bias, dtype=np.float32)
    proj_weight = np.asarray(proj_weight, dtype=np.float32)
    proj_bias = np.asarray(proj_bias, dtype=np.float32)
    gn_weight = np.asarray(gn_weight, dtype=np.float32)
    gn_bias = np.asarray(gn_bias, dtype=np.float32)

    b = x.shape[0]
    xf = x.reshape(b, C, N)
    wqT = np.ascontiguousarray(qkv_weight[0:C].T)
    wkT = np.ascontiguousarray(qkv_weight[C:2 * C].T)
    wvT = np.ascontiguousarray(qkv_weight[2 * C:3 * C].T)
    wpT = np.ascontiguousarray(proj_weight.T)
    qkvb = np.ascontiguousarray(qkv_bias.reshape(3, C).T)
    pbv = np.ascontiguousarray(proj_bias.reshape(C, 1))
    idt_np = np.eye(C, dtype=np.float32)
    gnwv = np.ascontiguousarray(gn_weight.reshape(C, 1))
    gnbv = np.ascontiguousarray(gn_bias.reshape(C, 1))

    in_maps = []
    for core in range(8):
        bi, half = core // 2, core % 2
        in_maps.append({
            "x": np.ascontiguousarray(xf[bi]),
            "xq": np.ascontiguousarray(xf[bi][:, half * NH:(half + 1) * NH]),
            "wqT": wqT, "wkT": wkT, "wvT": wvT, "wpT": wpT,
            "qkvb": qkvb, "pb": pbv, "gnw": gnwv, "gnb": gnbv,
            "idt": idt_np,
        })
    return in_maps


def run_on_cores(trace=False, reps=1, **inputs):
    """Build + run on the 8 cores; returns (BassKernelResults, output array)."""
    nc = _get_nc(reps)
    in_maps = _make_in_maps(**inputs)
    res = run_bass_kernel_spmd(nc, in_maps, core_ids=list(range(8)),
                               trace=trace)
    b = np.asarray(inputs["x"]).shape[0]
    h = w = 64
    out = np.empty((b, C, N), dtype=np.float32)
    for core in range(8):
        bi, half = core // 2, core % 2
        out[bi][:, half * NH:(half + 1) * NH] = res.results[core]["out"]
    return res, out.reshape(b, C, h, w)


def kernel(**inputs) -> np.ndarray:
    _, out = run_on_cores(trace=False, **inputs)
    return out
